# revision 1
# baseline (speedup 1.0000x reference)
"""Multi-head attention Trainium2 kernel (8 NeuronCores).

Sharding: core c = b*4 + g handles batch b (of 2) and head-group g (4 of the
16 heads). Q/K/V projections are column-sharded (256 cols per core), the
output projection is row-sharded; per-core partial outputs are summed on the
host (the all-reduce of a row-parallel matmul).

Per-core algorithm (all matmuls bf16 with f32 PSUM accumulation):
  Q.T = (WQg/8).T @ x_q.T           [256, 2048]  (scale 1/sqrt(D) folded in)
  K.T = WKg.T @ x_k.T               [256, 2048]
  V   = x_v @ WVg (+ ones column)   [2048, 4*65]
  S.T = Kh @ Qh.T per head          [Lk, Lq] tiles, 2 heads row-tiled on PE
  E.T = exp(S.T) * maskT            (multiplicative 0/1 mask == where(mask==0,-1e9) + softmax)
  Z.T|denom = V'.T @ E.T            M=65 matmul; row 64 = softmax denominator
  Z.T_norm = Z.T * bcast(1/denom)
  out_partial = Z.T_norm.T @ WOg    [2048, 1024] f32
Host: out[b] = sum_g out_partial[b,g] + bO.
"""

import sys
import types

sys.path.insert(0, "/opt/trn_rl_repo")

# The NTFF profiling hook module is absent in this container; shim it so
# run_bass_kernel_spmd(trace=True) degrades gracefully instead of crashing.
if "antenv.axon_hooks" not in sys.modules:
    _shim = types.ModuleType("antenv.axon_hooks")
    _shim.get_axon_ntff_profile_hook = lambda: None
    sys.modules["antenv.axon_hooks"] = _shim

import numpy as np
import ml_dtypes

import concourse.bass as bass
import concourse.mybir as mybir
import concourse.tile as tile
from concourse import bacc
from concourse.bass_utils import run_bass_kernel_spmd

BF16 = mybir.dt.bfloat16
F32 = mybir.dt.float32
AF = mybir.ActivationFunctionType
ALU = mybir.AluOpType

N_CORES = 8
B, L, C, H = 2, 2048, 1024, 16
D = C // H          # 64 head dim
G = 4               # head-groups per batch (cores per batch)
HPG = H // G        # 4 heads per group
DG = HPG * D        # 256 features per group
DP = D + 1          # head dim + ones column

P = 128
LQB = 512           # Lq block (psum free dim)
N_LQB = L // LQB    # 4
N_LK = L // P       # 16 Lk tiles
N_KT = C // P       # 8 contraction tiles for projections
MT_Q = DG // P      # 2 partition tiles for Q.T/K.T

_cached_nc = None
# set TRACE=True before calling kernel() to request an NTFF-profiled run
# (requires the axon NTFF hook; degrades gracefully without it). The
# measured per-core exec time lands in LAST_EXEC_NS.
TRACE = False
LAST_EXEC_NS = None


def _build():
    import os
    nc = bacc.Bacc("TRN2", target_bir_lowering=False, debug=False,
                   num_devices=N_CORES)

    xq = nc.dram_tensor("xq", [C, L], BF16, kind="ExternalInput").ap()
    xk = nc.dram_tensor("xk", [C, L], BF16, kind="ExternalInput").ap()
    xv = nc.dram_tensor("xv", [C, L], BF16, kind="ExternalInput").ap()
    wq = nc.dram_tensor("wq", [C, DG], BF16, kind="ExternalInput").ap()
    wk = nc.dram_tensor("wk", [C, DG], BF16, kind="ExternalInput").ap()
    wv = nc.dram_tensor("wv", [C, DG], BF16, kind="ExternalInput").ap()
    wo = nc.dram_tensor("wo", [DG, C], BF16, kind="ExternalInput").ap()
    bq = nc.dram_tensor("bq", [P, MT_Q], F32, kind="ExternalInput").ap()
    bk = nc.dram_tensor("bk", [P, MT_Q], F32, kind="ExternalInput").ap()
    bv = nc.dram_tensor("bv", [1, DG], BF16, kind="ExternalInput").ap()
    maskT = nc.dram_tensor("maskT", [L, L], BF16, kind="ExternalInput").ap()
    out_dt = BF16 if os.environ.get("K_OUT16", "1") == "1" else F32
    out = nc.dram_tensor("out", [L, C], out_dt, kind="ExternalOutput").ap()

    with tile.TileContext(nc) as tc:
        _body(tc, xq, xk, xv, wq, wk, wv, wo, bq, bk, bv, maskT, out)
    nc.compile()
    return nc


def _body(tc, xq, xk, xv, wq, wk, wv, wo, bq, bk, bv, maskT, out):
    import os
    PHASE = int(os.environ.get("K_PHASE", "99"))  # debug: truncate build
    nc = tc.nc
    from contextlib import ExitStack
    with ExitStack() as ctx:
        const = ctx.enter_context(tc.tile_pool(name="const", bufs=1))
        wpool = ctx.enter_context(tc.tile_pool(name="wpool", bufs=1))
        xvp = ctx.enter_context(tc.tile_pool(name="xvp", bufs=3))
        xs = ctx.enter_context(tc.tile_pool(name="xs", bufs=int(os.environ.get("K_XS", "5"))))
        qk = ctx.enter_context(tc.tile_pool(name="qk", bufs=1))
        vpool = ctx.enter_context(tc.tile_pool(name="vpool", bufs=1))
        ztp = ctx.enter_context(tc.tile_pool(name="ztp", bufs=1))
        ep = ctx.enter_context(tc.tile_pool(name="ep", bufs=int(os.environ.get("K_EP", "2"))))
        mp = ctx.enter_context(tc.tile_pool(name="mp", bufs=int(os.environ.get("K_MP", "2"))))
        op = ctx.enter_context(tc.tile_pool(name="op", bufs=int(os.environ.get("K_OP", "4"))))
        nrm = ctx.enter_context(tc.tile_pool(name="nrm", bufs=int(os.environ.get("K_NRM", "3"))))
        ps_big = ctx.enter_context(tc.tile_pool(
            name="ps_big", bufs=int(os.environ.get("K_PSB", "2")), space="PSUM"))
        ps_av = ctx.enter_context(tc.tile_pool(
            name="ps_av", bufs=int(os.environ.get("K_PSAV", "2")), space="PSUM"))
        OP_SHARE = os.environ.get("K_OPSHARE", "0")
        if OP_SHARE == "av":
            ps_op = ps_av
        elif OP_SHARE == "ps":
            ps_op = ps_big
        else:
            ps_op = ctx.enter_context(
                tc.tile_pool(name="ps_op", bufs=2, space="PSUM"))

        if os.environ.get("K_PROJPS", "op") == "op":
            proj_pool, proj_tag = ps_op, "ps_o"
        else:
            proj_pool, proj_tag = ps_big, "ps"

        # ---- resident constants / weights ----
        wq_sb = wpool.tile([P, N_KT, DG], BF16)
        wk_sb = wpool.tile([P, N_KT, DG], BF16)
        if int(os.environ.get("K_ORDER", "5")) in (7, 9, 10, 11, 12) or os.environ.get("K_KFIRST", "0") == "1":
            nc.sync.dma_start(wk_sb[:], wk.rearrange("(kt p) m -> p kt m", p=P))
            nc.sync.dma_start(wq_sb[:], wq.rearrange("(kt p) m -> p kt m", p=P))
        else:
            nc.sync.dma_start(wq_sb[:], wq.rearrange("(kt p) m -> p kt m", p=P))
            nc.sync.dma_start(wk_sb[:], wk.rearrange("(kt p) m -> p kt m", p=P))
        wv_sb = wpool.tile([P, N_KT, DG], BF16)
        wo_sb = wpool.tile([P, MT_Q, C], BF16)
        bq_sb = const.tile([P, MT_Q], F32)
        nc.sync.dma_start(bq_sb[:], bq[:])
        bk_sb = const.tile([P, MT_Q], F32)
        nc.sync.dma_start(bk_sb[:], bk[:])
        bv_sb = const.tile([1, DG], BF16)
        nc.sync.dma_start(bv_sb[:], bv[:])
        ones_sb = const.tile([1, P], BF16)
        nc.gpsimd.memset(ones_sb[:], 1.0)
        if os.environ.get("K_WARMEXP", "0") == "1":
            # warm the ACT exp table during the DMA-bound head so the first
            # real exp doesn't pay the lazy ACT_TABLE_LOAD
            warm = const.tile([1, 2], F32)
            nc.gpsimd.memset(warm[:], 0.0)
            nc.scalar.activation(warm[:], warm[:], AF.Exp)


        # ---- projection emitters ----
        qt_sb = qk.tile([P, MT_Q, L], BF16)
        kt_sb = qk.tile([P, MT_Q, L], BF16)
        v_sb = vpool.tile([P, N_LK, HPG, DP], BF16)
        # ones columns (written once; tile has bufs=1 so it is never recycled)
        nc.gpsimd.memset(v_sb[:, :, :, D:DP], 1.0)

        def emit_proj_part(x_dram, w_sb, b_sb, dst, mt, lqs):
            xr = x_dram.rearrange("(kt p) l -> p kt l", p=P)
            if True:
                for lq in lqs:
                    xt = xs.tile([P, N_KT, LQB], BF16, tag="xs", name="xs_t")
                    nc.sync.dma_start(
                        xt[:], xr[:, :, lq * LQB:(lq + 1) * LQB])
                    ps = proj_pool.tile([P, LQB], F32, tag=proj_tag,
                                        name="ps_proj")
                    for kt in range(N_KT):
                        nc.tensor.matmul(
                            ps[:], w_sb[:, kt, mt * P:(mt + 1) * P],
                            xt[:, kt, :],
                            start=(kt == 0), stop=(kt == N_KT - 1))
                    # psum -> sbuf bf16 with per-partition bias
                    if os.environ.get("K_QKEV", "dve") == "act":
                        nc.scalar.activation(
                            dst[:, mt, lq * LQB:(lq + 1) * LQB], ps[:],
                            AF.Identity, bias=b_sb[:, mt:mt + 1])
                    else:
                        nc.vector.tensor_scalar_add(
                            dst[:, mt, lq * LQB:(lq + 1) * LQB], ps[:],
                            b_sb[:, mt:mt + 1])

        def emit_qk_proj(mt):
            # K first: scores need all of K.T(mt) but only one Q.T lq-block,
            # so K's 4 slabs gate the first exp — front-load them
            if os.environ.get("K_KFIRST", "0") == "1":
                emit_proj_part(xk, wk_sb, bk_sb, kt_sb, mt, range(N_LQB))
                emit_proj_part(xq, wq_sb, bq_sb, qt_sb, mt, range(N_LQB))
            else:
                emit_proj_part(xq, wq_sb, bq_sb, qt_sb, mt, range(N_LQB))
                emit_proj_part(xk, wk_sb, bk_sb, kt_sb, mt, range(N_LQB))

        def emit_proj_slab(x_dram, w_sb, b_sb, dst, lqs):
            # one slab load serves both mt tiles (halves Q/K input DMA)
            xr = x_dram.rearrange("(kt p) l -> p kt l", p=P)
            for lq in lqs:
                xt = xs.tile([P, N_KT, LQB], BF16, tag="xs", name="xs_t")
                nc.sync.dma_start(
                    xt[:], xr[:, :, lq * LQB:(lq + 1) * LQB])
                for mt in range(MT_Q):
                    ps = proj_pool.tile([P, LQB], F32, tag=proj_tag,
                                        name="ps_proj")
                    for kt in range(N_KT):
                        nc.tensor.matmul(
                            ps[:], w_sb[:, kt, mt * P:(mt + 1) * P],
                            xt[:, kt, :],
                            start=(kt == 0), stop=(kt == N_KT - 1))
                    nc.vector.tensor_scalar_add(
                        dst[:, mt, lq * LQB:(lq + 1) * LQB], ps[:],
                        b_sb[:, mt:mt + 1])

        def emit_v_proj():
            nc.sync.dma_start(wv_sb[:], wv.rearrange("(kt p) m -> p kt m", p=P))
            xvr = xv.rearrange("(kt p) l -> p kt l", p=P)
            for lb in range(N_LQB):
                xt = xs.tile([P, N_KT, LQB], BF16, tag="xs", name="xs_t")
                nc.sync.dma_start(xt[:], xvr[:, :, lb * LQB:(lb + 1) * LQB])
                for sub in range(LQB // P):
                    mt = lb * (LQB // P) + sub
                    ps = proj_pool.tile([P, DG], F32, tag=proj_tag,
                                        name="ps_v")
                    for kt in range(N_KT):
                        nc.tensor.matmul(
                            ps[:], xt[:, kt, sub * P:(sub + 1) * P],
                            wv_sb[:, kt, :],
                            start=(kt == 0), stop=False)
                    # bias via rank-1 update: ones[1,128].T @ bv[1,256]
                    nc.tensor.matmul(ps[:], ones_sb[:], bv_sb[:],
                                     start=False, stop=True)
                    # evict: [128, 4, 64] strided into the padded layout
                    if os.environ.get("K_VEV", "dve") == "act":
                        nc.scalar.copy(
                            v_sb[:, mt, :, 0:D],
                            ps[:].rearrange("p (h d) -> p h d", h=HPG))
                    else:
                        nc.vector.tensor_copy(
                            v_sb[:, mt, :, 0:D],
                            ps[:].rearrange("p (h d) -> p h d", h=HPG))

        if PHASE < 2:
            emit_qk_proj(0)
            emit_qk_proj(1)
            emit_v_proj()
            return
        # ---- attention (software-pipelined: scores(i+1) emitted before
        # AV(i) so the in-order PE stream never stalls on exp/mask) ----
        zt_sb = ztp.tile([P, MT_Q, L], BF16)
        mask_tiles = {}

        MHALF = os.environ.get("K_MHALF", "1") == "1"

        def emit_scores(lq, pair):
            if lq not in mask_tiles:
                if MHALF:
                    halves = []
                    for hv in range(2):
                        m_sb = mp.tile([P, N_LK // 2, LQB], BF16,
                                       tag="mask", name="m_sb")
                        nc.sync.dma_start(
                            m_sb[:],
                            maskT[hv * (L // 2):(hv + 1) * (L // 2),
                                  lq * LQB:(lq + 1) * LQB]
                            .rearrange("(lk p) q -> p lk q", p=P))
                        halves.append(m_sb)
                    mask_tiles[lq] = halves
                else:
                    m_sb = mp.tile([P, N_LK, LQB], BF16, tag="mask",
                                   name="m_sb")
                    nc.sync.dma_start(
                        m_sb[:], maskT[:, lq * LQB:(lq + 1) * LQB]
                        .rearrange("(lk p) q -> p lk q", p=P))
                    mask_tiles[lq] = m_sb
            e_sb = ep.tile([P, N_LK, 2, LQB], BF16, tag="e", name="e_sb")
            # scores + exp, two heads row-tiled per lk tile
            for lk in range(N_LK):
                ps = ps_big.tile([P, 2 * LQB], F32, tag="ps", name="ps_s")
                nc.tensor.matmul(
                    ps[:, 0:LQB],
                    kt_sb[0:D, pair, lk * P:(lk + 1) * P],
                    qt_sb[0:D, pair, lq * LQB:(lq + 1) * LQB],
                    start=True, stop=True)
                nc.tensor.matmul(
                    ps[:, LQB:2 * LQB],
                    kt_sb[D:P, pair, lk * P:(lk + 1) * P],
                    qt_sb[D:P, pair, lq * LQB:(lq + 1) * LQB],
                    start=True, stop=True)
                nc.scalar.activation(e_sb[:, lk, :, :], ps[:], AF.Exp)
            # multiplicative mask (0/1): one DVE op for both heads via a
            # stride-0 broadcast of the mask over the head dim
            nparts = int(os.environ.get("K_MASK1", "8"))
            if nparts >= 2:
                # finer-grained mask ops let AV start before the whole
                # slab is masked
                CH = N_LK // nparts
                for hh in range(2):
                    for part in range(nparts):
                        sl = slice(part * CH, (part + 1) * CH)
                        if MHALF:
                            hv, base = divmod(part * CH, N_LK // 2)
                            msl = mask_tiles[lq][hv][:, base:base + CH, :]
                        else:
                            msl = mask_tiles[lq][:, sl, :]
                        nc.vector.tensor_tensor(
                            e_sb[:, sl, hh, :], e_sb[:, sl, hh, :],
                            msl, ALU.mult)
            elif nparts == 1:
                m_b = mask_tiles[lq][:, :, None, :].broadcast_to(
                    [P, N_LK, 2, LQB])
                nc.vector.tensor_tensor(e_sb[:], e_sb[:], m_b, ALU.mult)
            else:
                for hh in range(2):
                    nc.vector.tensor_tensor(
                        e_sb[:, :, hh, :], e_sb[:, :, hh, :],
                        mask_tiles[lq][:], ALU.mult)
            return e_sb

        def emit_av(lq, pair, e_sb):
            # A@V with fused denominator (ones column of V')
            for hh in range(2):
                h = pair * 2 + hh
                ps_z = ps_av.tile([P, LQB], F32, tag="av", name="ps_z")
                for lk in range(N_LK):
                    nc.tensor.matmul(
                        ps_z[0:DP, :],
                        v_sb[:, lk, h, :],
                        e_sb[:, lk, hh, :],
                        start=(lk == 0), stop=(lk == N_LK - 1))
                # normalize: Z.T[d, q] / denom[q]
                recip = nrm.tile([1, LQB], F32, tag="recip", name="recip")
                nc.vector.reciprocal(recip[:], ps_z[D:DP, :])
                bcast = nrm.tile([D, LQB], F32, tag="bcast", name="bcast")
                nc.gpsimd.partition_broadcast(bcast[:], recip[:])
                nc.vector.tensor_tensor(
                    zt_sb[hh * D:(hh + 1) * D, pair,
                          lq * LQB:(lq + 1) * LQB],
                    ps_z[0:D, :], bcast[:], ALU.mult)

        wo_loaded = [False]

        def emit_outproj(lq):
            # partial output rows for this lq block (needs both pairs' Z.T)
            if not wo_loaded[0]:
                nc.sync.dma_start(
                    wo_sb[:], wo.rearrange("(kt p) n -> p kt n", p=P))
                wo_loaded[0] = True
            for sub in range(LQB // P):
                mt = lq * (LQB // P) + sub
                o_dt = BF16 if os.environ.get("K_OUT16", "1") == "1" else F32
                o_sb = op.tile([P, C], o_dt, tag="o", name="o_sb")
                for nb in range(2):
                    optag = {"av": "av", "ps": "ps"}.get(OP_SHARE, "ps_o")
                    ps = ps_op.tile([P, LQB], F32, tag=optag, name="ps_o")
                    for kt in range(MT_Q):
                        nc.tensor.matmul(
                            ps[:], zt_sb[:, kt, mt * P:(mt + 1) * P],
                            wo_sb[:, kt, nb * LQB:(nb + 1) * LQB],
                            start=(kt == 0), stop=(kt == MT_Q - 1))
                    # eviction engine split is tunable; ACT idles in
                    # the tail but is the bottleneck overall
                    if nb == 0 or os.environ.get("K_OEV", "dve") == "dve":
                        nc.vector.tensor_copy(
                            o_sb[:, nb * LQB:(nb + 1) * LQB], ps[:])
                    else:
                        nc.scalar.copy(
                            o_sb[:, nb * LQB:(nb + 1) * LQB], ps[:])
                nc.sync.dma_start(out[mt * P:(mt + 1) * P, :], o_sb[:])

        # emission order: enough projection for the first scores blocks,
        # then a lag-1 pipeline of scores -> AV, with the output projection
        # of each lq block interleaved once both its pairs are normalized.
        ORDER = int(os.environ.get("K_ORDER", "5"))
        interleave_outproj = ORDER in (1, 2, 4, 5, 11, 14)
        # ORDER 6: phase order with an early first-scores block

        def attn_pipeline(start_pending, first):
            # `first` = number of blocks whose scores were already emitted
            pending = start_pending
            av_first = os.environ.get("K_AVFIRST", "0") == "1"
            for i in range(first, N_LQB * MT_Q):
                lq, pair = divmod(i, MT_Q)
                if av_first and pending:
                    blq, bpair, be = pending.pop(0)
                    emit_av(blq, bpair, be)
                    if interleave_outproj and bpair == MT_Q - 1:
                        emit_outproj(blq)
                e_sb = emit_scores(lq, pair)
                if not av_first and pending:
                    blq, bpair, be = pending.pop(0)
                    emit_av(blq, bpair, be)
                    if interleave_outproj and bpair == MT_Q - 1:
                        emit_outproj(blq)
                pending.append((lq, pair, e_sb))
            for (blq, bpair, be) in pending:
                emit_av(blq, bpair, be)
                if interleave_outproj and bpair == MT_Q - 1:
                    emit_outproj(blq)
            if not interleave_outproj:
                for lq in range(N_LQB):
                    emit_outproj(lq)

        if ORDER == 14:
            emit_qk_proj(0)
            e00 = emit_scores(0, 0)
            emit_qk_proj(1)
            emit_v_proj()
            e01 = emit_scores(0, 1)
            emit_av(0, 0, e00)
            attn_pipeline([(0, 1, e01)], first=2)
        elif ORDER == 13:
            emit_qk_proj(0)
            emit_qk_proj(1)
            e00 = emit_scores(0, 0)
            e01 = emit_scores(0, 1)
            e10 = emit_scores(1, 0)
            emit_v_proj()
            pend = [(0, 0, e00), (0, 1, e01), (1, 0, e10)]
            for i in range(3, N_LQB * MT_Q):
                lq, pair = divmod(i, MT_Q)
                blq, bpair, be = pend.pop(0)
                emit_av(blq, bpair, be)
                e_sb = emit_scores(lq, pair)
                pend.append((lq, pair, e_sb))
            for (blq, bpair, be) in pend:
                emit_av(blq, bpair, be)
            for lq in range(N_LQB):
                emit_outproj(lq)
        elif ORDER == 12:
            emit_proj_slab(xk, wk_sb, bk_sb, kt_sb, range(N_LQB))
            emit_proj_slab(xq, wq_sb, bq_sb, qt_sb, range(N_LQB))
            e00 = emit_scores(0, 0)
            e01 = emit_scores(0, 1)
            emit_v_proj()
            emit_av(0, 0, e00)
            attn_pipeline([(0, 1, e01)], first=2)
        elif ORDER == 11:
            emit_proj_slab(xk, wk_sb, bk_sb, kt_sb, range(N_LQB))
            emit_proj_slab(xq, wq_sb, bq_sb, qt_sb, [0])
            e00 = emit_scores(0, 0)
            emit_proj_slab(xq, wq_sb, bq_sb, qt_sb, [1, 2, 3])
            e01 = emit_scores(0, 1)
            e10 = emit_scores(1, 0)
            emit_v_proj()
            emit_av(0, 0, e00)
            emit_av(0, 1, e01)
            if interleave_outproj:
                emit_outproj(0)
            attn_pipeline([(1, 0, e10)], first=3)
        elif ORDER == 9:
            emit_proj_slab(xk, wk_sb, bk_sb, kt_sb, range(N_LQB))
            emit_proj_slab(xq, wq_sb, bq_sb, qt_sb, range(N_LQB))
            emit_v_proj()
            attn_pipeline([], first=0)
        elif ORDER == 10:
            emit_proj_slab(xk, wk_sb, bk_sb, kt_sb, [0])
            emit_proj_slab(xq, wq_sb, bq_sb, qt_sb, [0])
            e00 = emit_scores(0, 0)
            emit_proj_slab(xk, wk_sb, bk_sb, kt_sb, [1, 2, 3])
            emit_proj_slab(xq, wq_sb, bq_sb, qt_sb, [1, 2, 3])
            e01 = emit_scores(0, 1)
            emit_v_proj()
            emit_av(0, 0, e00)
            attn_pipeline([(0, 1, e01)], first=2)
        elif ORDER == 7:
            emit_proj_part(xk, wk_sb, bk_sb, kt_sb, 0, range(N_LQB))
            emit_proj_part(xq, wq_sb, bq_sb, qt_sb, 0, [0])
            e00 = emit_scores(0, 0)
            emit_proj_part(xq, wq_sb, bq_sb, qt_sb, 0, [1, 2, 3])
            emit_qk_proj(1)
            emit_v_proj()
            attn_pipeline([(0, 0, e00)], first=1)
        elif ORDER == 6:
            emit_qk_proj(0)
            e00 = emit_scores(0, 0)
            emit_qk_proj(1)
            emit_v_proj()
            attn_pipeline([(0, 0, e00)], first=1)
        elif ORDER == 5:
            emit_qk_proj(0)
            emit_qk_proj(1)
            e00 = emit_scores(0, 0)
            emit_v_proj()
            e01 = emit_scores(0, 1)
            emit_av(0, 0, e00)
            attn_pipeline([(0, 1, e01)], first=2)
        elif ORDER == 4:
            emit_qk_proj(0)
            emit_qk_proj(1)
            e00 = emit_scores(0, 0)
            e01 = emit_scores(0, 1)
            emit_v_proj()
            emit_av(0, 0, e00)
            attn_pipeline([(0, 1, e01)], first=2)
        elif ORDER == 3:
            emit_qk_proj(0)
            emit_qk_proj(1)
            e00 = emit_scores(0, 0)
            e01 = emit_scores(0, 1)
            emit_v_proj()
            emit_av(0, 0, e00)
            attn_pipeline([(0, 1, e01)], first=2)
        elif ORDER == 2:
            emit_qk_proj(0)
            e00 = emit_scores(0, 0)
            emit_qk_proj(1)
            e01 = emit_scores(0, 1)
            emit_v_proj()
            emit_av(0, 0, e00)
            attn_pipeline([(0, 1, e01)], first=2)
        else:
            emit_qk_proj(0)
            emit_qk_proj(1)
            emit_v_proj()
            attn_pipeline([], first=0)


def get_nc():
    global _cached_nc
    if _cached_nc is None:
        _cached_nc = _build()
    return _cached_nc


def _bf16(x):
    return np.asarray(x, dtype=np.float32).astype(ml_dtypes.bfloat16)


def kernel(**inputs):
    query = np.asarray(inputs["query"], np.float32)
    key = np.asarray(inputs["key"], np.float32)
    value = np.asarray(inputs["value"], np.float32)
    mask = np.asarray(inputs["mask"])
    WQ = np.asarray(inputs["WQ"], np.float32)
    bQ = np.asarray(inputs["bQ"], np.float32)
    WK = np.asarray(inputs["WK"], np.float32)
    bK = np.asarray(inputs["bK"], np.float32)
    WV = np.asarray(inputs["WV"], np.float32)
    bV = np.asarray(inputs["bV"], np.float32)
    WO = np.asarray(inputs["WO"], np.float32)
    bO = np.asarray(inputs["bO"], np.float32)

    nc = get_nc()

    scale = 1.0 / np.sqrt(np.float32(D))
    # per-batch host prep (shared across the 4 cores of a batch)
    xqT = [_bf16(query[b].T) for b in range(B)]
    xkT = [_bf16(key[b].T) for b in range(B)]
    xvT = [_bf16(value[b].T) for b in range(B)]
    maskTb = [_bf16(mask[b, 0].T) for b in range(B)]
    in_maps = []
    for c in range(N_CORES):
        b, g = divmod(c, G)
        sl = slice(g * DG, (g + 1) * DG)
        in_maps.append({
            "xq": xqT[b], "xk": xkT[b], "xv": xvT[b],
            "wq": _bf16(WQ[:, sl] * scale),
            "wk": _bf16(WK[:, sl]),
            "wv": _bf16(WV[:, sl]),
            "wo": _bf16(WO[sl, :]),
            "bq": np.ascontiguousarray(
                (bQ[sl] * scale).reshape(MT_Q, P).T).astype(np.float32),
            "bk": np.ascontiguousarray(
                bK[sl].reshape(MT_Q, P).T).astype(np.float32),
            "bv": _bf16(bV[sl]).reshape(1, DG),
            "maskT": maskTb[b],
        })

    global LAST_EXEC_NS
    res = run_bass_kernel_spmd(nc, in_maps, core_ids=list(range(N_CORES)),
                               trace=TRACE)
    LAST_EXEC_NS = res.exec_time_ns

    outp = np.zeros((B, L, C), np.float32)
    for c in range(N_CORES):
        b = c // G
        outp[b] += res.results[c]["out"].astype(np.float32)
    outp += bO.astype(np.float32)
    return outp

